# revision 7
# baseline (speedup 1.0000x reference)
"""Trainium2 Bass kernel for BatchRemoveQuatDiscontinuities (v2).

Algorithm (per (batch, joint) lane):
    d[t]    = dot(q[t], q[t-1])                (fp32, 4-wide dot)
    flip[t] = 1 if d[t] < 0 else 0             (t >= 1; flip[0] = 0)
    sigma[t] = (-1)^(sum_{s<=t} flip[s])       (cumulative sign parity)
    out[t]  = q[t] * sigma[t]

Mapping (data-parallel over batch across 8 cores, CPT clips per tile):
  * One tile = CPT batch clips, one fully-contiguous 2MB DMA per clip
    pair: [128 partitions = t/8, free = (clip, ts: 8, j: 64, c: 4)].
  * q[t-1] at octet boundaries (ts=0) comes from a TensorE matmul with
    an off-diagonal 0/1 matrix S into PSUM (fp32 exact), per clip chunk.
  * dot: product on DVE, two pairwise adds (engine-assignable); the
    second add writes d in (clip, j, ts) order for the scan.
  * flip indicator e = Relu(-d) on ScalarE (d == 0 -> no flip; the
    matmul's zero row 0 makes t=0 flip-free automatically).
  * Within-octet flip parity: tensor_tensor_scan xor with a reset mask
    (state = (mask*state) xor e), segments of 8 per (clip, joint).
    Octet-level flip count: strict-triangular matmul over partitions on
    the ts=7 parities; parity of the count via mod 2 (or int path).
  * sigma = (1-2*rowp) * (1-2*offr) in bf16 (exact +/-1), expanded over
    c by broadcast APs in the final multiply, which is split DVE/GpSimd
    by ts to balance engine load.
"""

import numpy as np
from contextlib import ExitStack

import concourse.bass as bass
import concourse.bacc as bacc
import concourse.tile as tile
from concourse import mybir
from concourse.bass_utils import run_bass_kernel_spmd

B, T, J, C = 128, 1024, 64, 4
NCORES = 8
JC = J * C                      # 256 floats per t
BPC = B // NCORES               # 16 batch clips per core
TS = 8                          # t per partition (octet)
FD = TS * JC                    # per-clip free dim = 2048 floats
SD = J * TS                     # per-clip prefix free dim = 512 (j, ts)
CLIP = T * JC                   # 262144 floats per clip

FP32 = mybir.dt.float32
BF16 = mybir.dt.bfloat16
I32 = mybir.dt.int32
Alu = mybir.AluOpType
Act = mybir.ActivationFunctionType


def _ap(apx, dims, extra_offset=0):
    """AP with explicit [step, count] free dims appended to partition dim."""
    return bass.AP(
        tensor=apx.tensor, offset=apx.offset + extra_offset,
        ap=[list(apx.ap[0]), *[list(d) for d in dims]],
    )


def build_nc(bpc=BPC, t=T, reps=1, mode="full", cpt=2, mult_split=2,
             u_eng="v", d_eng="v", sig_eng="v", st_eng="sync",
             parity="int"):
    assert t % (128 * TS) == 0 and t == T
    assert bpc % cpt == 0
    nt = bpc // cpt             # tiles per core
    fd = cpt * FD               # tile free dim
    sd = cpt * SD               # tile prefix dim
    nc = bacc.Bacc(None, target_bir_lowering=False)
    q = nc.declare_dram_parameter("q", [bpc, t, J, C], FP32, isOutput=False)
    smat = nc.declare_dram_parameter("smat", [128, 128], FP32, isOutput=False)
    pmat = nc.declare_dram_parameter("pmat", [128, 128], FP32, isOutput=False)
    out = nc.declare_dram_parameter("out", [bpc, t, J, C], FP32, isOutput=True)
    qf = q.rearrange("b t j c -> (b t j c)")
    of = out.rearrange("b t j c -> (b t j c)")

    def eng(key):
        return nc.vector if key == "v" else nc.gpsimd

    st = nc.sync if st_eng == "sync" else nc.scalar

    with tile.TileContext(nc) as tc, ExitStack() as ctx:
        consts = ctx.enter_context(tc.tile_pool(name="consts", bufs=1))
        qpool = ctx.enter_context(tc.tile_pool(name="qpool", bufs=4))
        opool = ctx.enter_context(tc.tile_pool(name="opool", bufs=4))
        bpool = ctx.enter_context(tc.tile_pool(name="bpool", bufs=2))
        spool = ctx.enter_context(tc.tile_pool(name="spool", bufs=3))
        auxp = ctx.enter_context(tc.tile_pool(name="auxp", bufs=2, space="PSUM"))
        offp = ctx.enter_context(tc.tile_pool(name="offp", bufs=3, space="PSUM"))

        smatSB = consts.tile([128, 128], FP32)
        nc.sync.dma_start(out=smatSB[:, :], in_=smat[:, :])
        pmatSB = consts.tile([128, 128], FP32)
        nc.sync.dma_start(out=pmatSB[:, :], in_=pmat[:, :])
        amask = consts.tile([128, sd], FP32)
        nc.vector.memset(amask[:, :], 1.0)
        nc.vector.memset(
            amask.rearrange("p (cl j ts) -> p cl j ts", ts=TS, j=J)[:, :, :, 0],
            0.0,
        )

        def hbm_ap(flat, n):
            return bass.AP(
                tensor=flat.tensor, offset=flat.offset + n * cpt * CLIP,
                ap=[[FD, 128], [CLIP, cpt], [1, FD]],
            )

        S = [dict() for _ in range(nt)]

        def ok(n):
            return 0 <= n < nt

        def e_load(n):
            if not ok(n):
                return
            S[n]["qt"] = qt = qpool.tile([128, fd], FP32, tag="qt", name="qt")
            nc.sync.dma_start(out=qt[:, :], in_=hbm_ap(qf, n))

        def e_aux(n):
            if not ok(n):
                return
            # octet-boundary shift: aux[p, (cl, jc)] = qt[p-1, cl, ts=7, jc]
            qt = S[n]["qt"]
            S[n]["aux"] = aux = auxp.tile([128, cpt * JC], FP32, tag="aux", name="aux")
            nc.tensor.matmul(
                aux[:, :],
                lhsT=smatSB[:, :],
                rhs=_ap(qt, [[FD, cpt], [1, JC]], FD - JC),
                start=True,
                stop=True,
            )

        def e_prods(n):
            if not ok(n):
                return
            # prod: o = q * q_shifted (hi: within-partition, lo: from aux)
            qt = S[n]["qt"]
            S[n]["o"] = o = opool.tile([128, fd], FP32, tag="o", name="o")
            nc.vector.tensor_tensor(
                out=_ap(o, [[FD, cpt], [1, FD - JC]], JC),
                in0=_ap(qt, [[FD, cpt], [1, FD - JC]], JC),
                in1=_ap(qt, [[FD, cpt], [1, FD - JC]], 0),
                op=Alu.mult,
            )
            nc.vector.tensor_tensor(
                out=_ap(o, [[FD, cpt], [1, JC]], 0),
                in0=_ap(qt, [[FD, cpt], [1, JC]], 0),
                in1=_ap(S[n]["aux"], [[JC, cpt], [1, JC]], 0),
                op=Alu.mult,
            )

        def e_u(n):
            if not ok(n):
                return
            # dot over c, pairwise: u[s, k] = o[4s+2k] + o[4s+2k+1]
            o = S[n]["o"]
            S[n]["u"] = u = bpool.tile([128, fd // 2], FP32, tag="u", name="u")
            eng(u_eng).tensor_tensor(
                out=_ap(u, [[2, fd // 4], [1, 2]], 0),
                in0=_ap(o, [[4, fd // 4], [2, 2]], 0),
                in1=_ap(o, [[4, fd // 4], [2, 2]], 1),
                op=Alu.add,
            )

        def e_d(n):
            if not ok(n):
                return
            # d written in (clip, j, ts) order for the segmented scan
            u = S[n]["u"]
            S[n]["d"] = d = bpool.tile([128, sd], FP32, tag="d", name="d")
            eng(d_eng).tensor_tensor(
                out=_ap(d, [[SD, cpt], [1, TS], [TS, J]], 0),
                in0=_ap(u, [[SD * 2, cpt], [J * 2, TS], [2, J]], 0),
                in1=_ap(u, [[SD * 2, cpt], [J * 2, TS], [2, J]], 1),
                op=Alu.add,
            )

        def e_relu(n):
            if not ok(n):
                return
            # flip indicator e = Relu(-d) in place (d == 0 -> no flip)
            d = S[n]["d"]
            nc.scalar.activation(d[:, :], d[:, :], Act.Relu, scale=-1.0)

        def e_scan(n):
            if not ok(n):
                return
            # within-octet inclusive flip parity (segmented xor-scan)
            S[n]["rowp"] = rowp = bpool.tile([128, sd], FP32, tag="rowp", name="rowp")
            nc.vector.tensor_tensor_scan(
                out=rowp[:, :], data0=amask[:, :], data1=S[n]["d"][:, :],
                initial=0.0, op0=Alu.mult, op1=Alu.logical_xor,
            )

        def e_offs(n):
            if not ok(n):
                return
            # octet-level: count of odd octets above (via matmul)
            S[n]["offs"] = offs = offp.tile([128, cpt * J], FP32, tag="offs", name="offs")
            nc.tensor.matmul(
                offs[:, :],
                lhsT=pmatSB[:, :],
                rhs=_ap(S[n]["rowp"], [[SD, cpt], [TS, J]], TS - 1),
                start=True,
                stop=True,
            )

        def e_sigr(n):
            if not ok(n):
                return
            S[n]["sigr"] = sigr = spool.tile([128, sd], BF16, tag="sigr", name="sigr")
            nc.scalar.activation(sigr[:, :], S[n]["rowp"][:, :], Act.Copy,
                                 bias=1.0, scale=-2.0)

        def e_mod(n):
            if not ok(n):
                return
            offs = S[n]["offs"]
            if parity == "mod":
                S[n]["offr"] = offr = spool.tile([128, cpt * J], FP32, tag="offr", name="offr")
                nc.vector.tensor_scalar(
                    out=offr[:, :], in0=offs[:, :], scalar1=2.0, scalar2=None,
                    op0=Alu.mod,
                )
            else:
                offi = spool.tile([128, cpt * J], I32, tag="offi", name="offi")
                nc.vector.tensor_copy(out=offi[:, :], in_=offs[:, :])
                S[n]["offr"] = offb = spool.tile([128, cpt * J], I32, tag="offb", name="offb")
                nc.vector.tensor_scalar(
                    out=offb[:, :], in0=offi[:, :], scalar1=1, scalar2=None,
                    op0=Alu.bitwise_and,
                )

        def e_sigo(n):
            if not ok(n):
                return
            S[n]["sigo"] = sigo = spool.tile([128, cpt * J], BF16, tag="sigo", name="sigo")
            nc.scalar.activation(sigo[:, :], S[n]["offr"][:, :], Act.Copy,
                                 bias=1.0, scale=-2.0)

        def e_sig(n):
            if not ok(n):
                return
            # sigma = sigma_row * sigma_off in (clip, j, ts) layout
            S[n]["sig"] = sig = spool.tile([128, sd], BF16, tag="sig", name="sig")
            eng(sig_eng).tensor_tensor(
                out=sig[:, :], in0=S[n]["sigr"][:, :],
                in1=_ap(S[n]["sigo"], [[J, cpt], [1, J], [0, TS]], 0),
                op=Alu.mult,
            )

        def e_finv(n):
            if not ok(n) or mult_split == 0:
                return
            qt, o, sig = S[n]["qt"], S[n]["o"], S[n]["sig"]
            ms = mult_split
            for cl in range(cpt):
                nc.vector.tensor_tensor(
                    out=_ap(o, [[JC, ms], [1, JC]], cl * FD),
                    in0=_ap(qt, [[JC, ms], [1, JC]], cl * FD),
                    in1=_ap(sig, [[1, ms], [TS, J], [0, C]], cl * SD),
                    op=Alu.mult,
                )

        def e_fing(n):
            if not ok(n) or mult_split == TS:
                return
            qt, o, sig = S[n]["qt"], S[n]["o"], S[n]["sig"]
            ms = mult_split
            for cl in range(cpt):
                nc.gpsimd.tensor_tensor(
                    out=_ap(o, [[JC, TS - ms], [1, JC]], cl * FD + ms * JC),
                    in0=_ap(qt, [[JC, TS - ms], [1, JC]], cl * FD + ms * JC),
                    in1=_ap(sig, [[1, TS - ms], [TS, J], [0, C]],
                            cl * SD + ms),
                    op=Alu.mult,
                )

        def e_store(n):
            if not ok(n):
                return
            st.dma_start(out=hbm_ap(of, n), in_=S[n]["o"][:, :])
            S[n].clear()    # drop tile refs so pool bufs recycle

        def emit_body():
            if mode == "dma":
                for n in range(nt):
                    S[n]["qt"] = qt = qpool.tile([128, fd], FP32, tag="qt", name="qt")
                    nc.sync.dma_start(out=qt[:, :], in_=hbm_ap(qf, n))
                    nc.sync.dma_start(out=hbm_ap(of, n), in_=qt[:, :])
                    S[n].clear()
                return
            # Software-pipelined emission: per-engine queue order matches
            # pipeline stage order across tiles, so no engine head-of-line
            # blocks the next tile's early work behind this tile's late
            # work.  Stage skews: load(k) | prods..scan(k-1) | parity/
            # sigma(k-2) | final mult + store(k-3).
            for k in range(nt + 3):
                e_load(k)
                e_aux(k - 1)
                e_prods(k - 1)
                e_u(k - 1)
                e_mod(k - 2)
                e_sigo(k - 2)
                e_sig(k - 2)
                e_finv(k - 3)
                e_fing(k - 3)
                e_store(k - 3)
                e_d(k - 1)
                e_relu(k - 1)
                e_scan(k - 1)
                e_offs(k - 1)
                e_sigr(k - 1)

        if reps == 1:
            emit_body()
        else:
            with tc.For_i(0, reps, 1):
                emit_body()
    return nc


def make_consts():
    smat = np.eye(128, k=1, dtype=np.float32)       # S[k, m] = 1 iff m == k+1
    pmat = np.triu(np.ones((128, 128), np.float32), k=1)  # strict prefix
    return smat, pmat


def kernel(joint_rotations: np.ndarray) -> np.ndarray:
    q = np.ascontiguousarray(joint_rotations, dtype=np.float32)
    assert q.shape == (B, T, J, C)
    smat, pmat = make_consts()
    nc = build_nc()
    nc.finalize()   # run bacc passes (wait splitting, reg alloc) + freeze
    in_maps = [
        {"q": q[c * BPC:(c + 1) * BPC], "smat": smat, "pmat": pmat}
        for c in range(NCORES)
    ]
    res = run_bass_kernel_spmd(nc, in_maps, list(range(NCORES)))
    outs = [np.asarray(r["out"]) for r in res.results]
    return np.concatenate(outs, axis=0)
